# revision 5
# baseline (speedup 1.0000x reference)
"""Causal self-attention kernel for 8 Trainium2 NeuronCores.

Problem: B=2, T=2048, D=2048, H=16, Dh=128, fp32.
  qkv = x @ Wqkv + bqkv ; per-head causal attention ; out = att @ Wout + bout

Sharding (tensor parallel over heads + AllToAll before out_proj):
  Core c owns heads {2c, 2c+1}. Each core computes, for all 4096 tokens,
  Q^T/K^T (head-dim on partitions) and V (token-dim on partitions) for its
  two heads via the QKV projection with its 768-column shard of Wqkv, runs
  causal attention locally (scores are computed transposed: S^T[k,q], so
  softmax reduction over k maps to a ones-matmul on the partition axis),
  and produces att^T [256, 4096]. An AllToAll redistributes from
  head-sharded to token-sharded: core c ends with att_all^T [2048, 512] for
  tokens [512c, 512c+512), projects with the full Wout, and returns its
  512-token slice of the output. The host concatenates the 8 slices.

All matmuls run in float32r (full PE rate at free-dim >= 256, ~1e-4 rel
error). PSUM accumulation is fp32.
"""

import numpy as np

import concourse.bass as bass
import concourse.mybir as mybir
import concourse.tile as tile
from concourse import bacc
from concourse.bass_utils import run_bass_kernel_spmd

B, T, D, H, Dh = 2, 2048, 2048, 16, 128
NT = B * T                  # 4096 tokens total
W = 8                       # cores
HL = H // W                 # 2 heads per core
CQKV = 3 * HL * Dh          # 768 qkv columns per core
KO = D // 128               # 16 contraction subtiles
TC = 256                    # token chunk for projection rhs
NTC = NT // TC              # 16 chunks
QC = 512                    # attention q-chunk
NQC = T // QC               # 4 q-chunks per batch
TOK = NT // W               # 512 tokens owned per core after AllToAll
SCALE = 1.0 / float(np.sqrt(Dh))

F32 = mybir.dt.float32
F32R = mybir.dt.float32r


def _build():
    nc = bacc.Bacc("TRN2", target_bir_lowering=False, debug=False,
                   enable_asserts=True, num_devices=W)
    xT = nc.dram_tensor("xT", [D, NT], F32, kind="ExternalInput").ap()
    wqkv = nc.dram_tensor("wqkv", [D, CQKV], F32, kind="ExternalInput").ap()
    bqkv = nc.dram_tensor("bqkv", [CQKV], F32, kind="ExternalInput").ap()
    wout = nc.dram_tensor("wout", [D, D], F32, kind="ExternalInput").ap()
    bout = nc.dram_tensor("bout", [D], F32, kind="ExternalInput").ap()
    masks = nc.dram_tensor("masks", [4, 128, QC], F32, kind="ExternalInput").ap()
    ones = nc.dram_tensor("ones", [128, 128], F32, kind="ExternalInput").ap()
    bvbc = nc.dram_tensor("bvbc", [128, HL * Dh], F32, kind="ExternalInput").ap()
    boutbc = nc.dram_tensor("boutbc", [128, D], F32, kind="ExternalInput").ap()
    out = nc.dram_tensor("out", [TOK, D], F32, kind="ExternalOutput").ap()

    xT_v = xT.rearrange("(ko p) t -> p ko t", p=128)
    wqkv_v = wqkv.rearrange("(ko p) c -> p ko c", p=128)
    wout_v = wout.rearrange("(ko p) c -> p ko c", p=128)

    with tile.TileContext(nc) as tc:
        # ---- small persistent constants + DRAM bounce buffers ----
        with tc.tile_pool(name="persist", bufs=1) as persist, \
             tc.tile_pool(name="dram", bufs=1, space="DRAM") as dram_pool:
            mask_sb = persist.tile([128, 4, QC], F32R)
            ones_sb = persist.tile([128, 128], F32R)
            bqk_sb = persist.tile([128, 2 * HL], F32)      # Q,K bias (col on partition)
            bv_sb = persist.tile([128, HL * Dh], F32)      # V bias pre-broadcast
            bout_sb = persist.tile([128, D], F32)          # out bias pre-broadcast

            nc.sync.dma_start(mask_sb[:], masks.rearrange("o p q -> p o q").bitcast(F32R))
            nc.sync.dma_start(ones_sb[:], ones.bitcast(F32R))
            nc.sync.dma_start(bqk_sb[:], bqkv[0:2 * HL * 128].rearrange("(cc p) -> p cc", p=128))
            nc.sync.dma_start(bv_sb[:], bvbc)
            nc.sync.dma_start(bout_sb[:], boutbc)

            a2a_in = dram_pool.tile([W, HL * 128, TOK], F32)
            a2a_out = dram_pool.tile([W, HL * 128, TOK], F32)

            with tc.tile_pool(name="qkv_pool", bufs=1) as qkv_pool:
                qT_sb = qkv_pool.tile([128, HL, NT], F32R)            # 32 KB/part
                kT_sb = qkv_pool.tile([128, HL, NT], F32R)            # 32 KB/part
                v_sb = qkv_pool.tile([128, HL, NT // 128, Dh], F32R)  # 32 KB/part

                # ---- phase 1: QKV projection ----
                with tc.tile_pool(name="wq_pool", bufs=1) as wq_pool, \
                     tc.tile_pool(name="x_pool", bufs=2) as x_pool, \
                     tc.tile_pool(name="proj_psum", bufs=4, space="PSUM") as proj_psum:
                    wqkv_sb = wq_pool.tile([128, KO, CQKV], F32R)     # 48 KB/part
                    nc.sync.dma_start(wqkv_sb[:], wqkv_v.bitcast(F32R))

                    for tci in range(NTC):
                        x_sb = x_pool.tile([128, KO, TC], F32R, name="x_sb")
                        nc.sync.dma_start(x_sb[:], xT_v[:, :, tci * TC:(tci + 1) * TC].bitcast(F32R))
                        # Q^T and K^T: weight-stationary, out [col(128), tok(TC)]
                        for cc in range(2 * HL):
                            ps = proj_psum.tile([128, TC], F32, name="proj_ps")
                            for ko in range(KO):
                                nc.tensor.matmul(
                                    ps[:], wqkv_sb[:, ko, cc * 128:(cc + 1) * 128],
                                    x_sb[:, ko, :], start=(ko == 0), stop=(ko == KO - 1))
                            dest = qT_sb if cc < HL else kT_sb
                            hl = cc if cc < HL else cc - HL
                            nc.vector.tensor_scalar_add(
                                dest[:, hl, tci * TC:(tci + 1) * TC], ps[:],
                                bqk_sb[:, cc:cc + 1])
                        # V: x-stationary, out [tok(128), col(HL*Dh)] - natural layout
                        for tb in range(TC // 128):
                            ps = proj_psum.tile([128, HL * Dh], F32, name="projv_ps")
                            for ko in range(KO):
                                nc.tensor.matmul(
                                    ps[:], x_sb[:, ko, tb * 128:(tb + 1) * 128],
                                    wqkv_sb[:, ko, 2 * HL * 128:], start=(ko == 0), stop=(ko == KO - 1))
                            idx = tci * (TC // 128) + tb
                            nc.vector.tensor_tensor(
                                v_sb[:, :, idx, :],
                                ps[:].rearrange("p (hl d) -> p hl d", hl=HL),
                                bv_sb[:].rearrange("p (hl d) -> p hl d", hl=HL),
                                mybir.AluOpType.add)

                # ---- phase 2: causal attention (scores computed transposed) ----
                with tc.tile_pool(name="att_pool", bufs=1) as att_pool:
                    att_sb = att_pool.tile([128, HL, NT], F32)        # 32 KB/part
                    with tc.tile_pool(name="ex_pool", bufs=3) as ex_pool, \
                         tc.tile_pool(name="den_pool", bufs=2) as den_pool, \
                         tc.tile_pool(name="rden_pool", bufs=2) as rden_pool, \
                         tc.tile_pool(name="s_psum", bufs=3, space="PSUM") as s_psum, \
                         tc.tile_pool(name="av_psum", bufs=2, space="PSUM") as av_psum, \
                         tc.tile_pool(name="d_psum", bufs=2, space="PSUM") as d_psum:
                        for b in range(B):
                            for hl in range(HL):
                                for qc in range(NQC):
                                    q0 = b * T + qc * QC
                                    nkb = (qc + 1) * (QC // 128)
                                    ps_av = av_psum.tile([128, QC], F32, name="ps_av")
                                    den = den_pool.tile([128, QC], F32R, name="den")
                                    for kb in range(nkb):
                                        k0 = b * T + kb * 128
                                        ps_s = s_psum.tile([128, QC], F32, name="ps_s")
                                        nc.tensor.matmul(
                                            ps_s[:], kT_sb[:, hl, k0:k0 + 128],
                                            qT_sb[:, hl, q0:q0 + QC], start=True, stop=True)
                                        ex = ex_pool.tile([128, QC], F32R, name="ex")
                                        nc.scalar.activation(
                                            ex[:], ps_s[:], mybir.ActivationFunctionType.Exp,
                                            scale=SCALE)
                                        o = kb - qc * (QC // 128)
                                        if o >= 0:
                                            nc.vector.tensor_tensor(
                                                ex[:], ex[:], mask_sb[:, o, :], mybir.AluOpType.mult)
                                        if kb == 0:
                                            nc.vector.tensor_copy(den[:], ex[:])
                                        else:
                                            nc.vector.tensor_tensor(
                                                den[:], den[:], ex[:], mybir.AluOpType.add)
                                        nc.tensor.matmul(
                                            ps_av[:], v_sb[:, hl, b * (T // 128) + kb, :], ex[:],
                                            start=(kb == 0), stop=(kb == nkb - 1))
                                    ps_dbc = d_psum.tile([128, QC], F32, name="ps_dbc")
                                    nc.tensor.matmul(ps_dbc[:], ones_sb[:], den[:],
                                                     start=True, stop=True)
                                    rden = rden_pool.tile([128, QC], F32, name="rden")
                                    nc.vector.reciprocal(rden[:], ps_dbc[:])
                                    nc.vector.tensor_tensor(
                                        att_sb[:, hl, q0:q0 + QC], ps_av[:],
                                        rden[:], mybir.AluOpType.mult)

                    # ---- phase 3: AllToAll head-sharded -> token-sharded ----
                    for r in range(W):
                        nc.sync.dma_start(
                            a2a_in[r].rearrange("(hl p) t -> p hl t", hl=HL, p=128),
                            att_sb[:, :, r * TOK:(r + 1) * TOK])
                    nc.gpsimd.collective_compute(
                        "AllToAll", mybir.AluOpType.bypass,
                        replica_groups=[list(range(W))],
                        ins=[a2a_in[:].opt()], outs=[a2a_out[:].opt()])

            # ---- phase 4: output projection for own 512 tokens ----
            with tc.tile_pool(name="attall_pool", bufs=1) as attall_pool, \
                 tc.tile_pool(name="wout_pool", bufs=2) as wout_pool, \
                 tc.tile_pool(name="o_pool", bufs=3) as o_pool, \
                 tc.tile_pool(name="out_psum", bufs=4, space="PSUM") as out_psum:
                attall_sb = attall_pool.tile([128, KO, TOK], F32R)
                nc.sync.dma_start(
                    attall_sb[:],
                    a2a_out[:].rearrange("r (x p) t -> p (r x) t", x=HL, p=128).bitcast(F32R))
                for colc in range(D // 512):
                    wout_sb = wout_pool.tile([128, KO, 512], F32R, name="wout_sb")
                    nc.sync.dma_start(
                        wout_sb[:], wout_v[:, :, colc * 512:(colc + 1) * 512].bitcast(F32R))
                    for tokc in range(TOK // 128):
                        ps_o = out_psum.tile([128, 512], F32, name="ps_o")
                        for ko in range(KO):
                            nc.tensor.matmul(
                                ps_o[:], attall_sb[:, ko, tokc * 128:(tokc + 1) * 128],
                                wout_sb[:, ko, :], start=(ko == 0), stop=(ko == KO - 1))
                        o_sb = o_pool.tile([128, 512], F32, name="o_sb")
                        nc.vector.tensor_tensor(
                            o_sb[:], ps_o[:],
                            bout_sb[:, colc * 512:(colc + 1) * 512],
                            mybir.AluOpType.add)
                        nc.sync.dma_start(
                            out[tokc * 128:(tokc + 1) * 128, colc * 512:(colc + 1) * 512],
                            o_sb[:])
    nc.compile()
    return nc


def _host_masks():
    m = np.zeros((4, 128, QC), np.float32)
    for o in range(4):
        kl = np.arange(128)[:, None]
        ql = np.arange(QC)[None, :]
        m[o] = (ql >= kl + 128 * o).astype(np.float32)
    return m


_CACHED_NC = None


def kernel(x, Wqkv, bqkv, Wout, bout):
    global _CACHED_NC
    x = np.asarray(x, dtype=np.float32)
    Wqkv = np.asarray(Wqkv, dtype=np.float32)
    bqkv = np.asarray(bqkv, dtype=np.float32)
    Wout = np.asarray(Wout, dtype=np.float32)
    bout = np.asarray(bout, dtype=np.float32)

    if _CACHED_NC is None:
        _CACHED_NC = _build()
    nc = _CACHED_NC

    xT = np.ascontiguousarray(x.reshape(NT, D).T)          # [D, NT]
    wq4 = Wqkv.reshape(D, 3, H, Dh)                        # col = which, head, dh
    bq4 = bqkv.reshape(3, H, Dh)
    masks = _host_masks()

    in_maps = []
    for c in range(W):
        wshard = np.ascontiguousarray(
            wq4[:, :, 2 * c:2 * c + HL, :].reshape(D, CQKV))
        bshard = np.ascontiguousarray(
            bq4[:, 2 * c:2 * c + HL, :].reshape(CQKV))
        in_maps.append({
            "xT": xT, "wqkv": wshard, "bqkv": bshard,
            "wout": Wout, "bout": bout, "masks": masks,
            "ones": np.ones((128, 128), np.float32),
            "bvbc": np.tile(bshard[2 * HL * 128:][None, :], (128, 1)),
            "boutbc": np.tile(bout[None, :], (128, 1)),
        })

    res = run_bass_kernel_spmd(nc, in_maps, core_ids=list(range(W)))
    full = np.concatenate([res.results[c]["out"] for c in range(W)], axis=0)
    return full.reshape(B, T, D)


# revision 7
# speedup vs baseline: 1.1843x; 1.1843x over previous
"""Causal self-attention kernel for 8 Trainium2 NeuronCores.

Problem: B=2, T=2048, D=2048, H=16, Dh=128, fp32.
  qkv = x @ Wqkv + bqkv ; per-head causal attention ; out = att @ Wout + bout

Sharding (tensor parallel over heads + AllToAll before out_proj):
  Core c owns heads {2c, 2c+1}. Each core computes, for all 4096 tokens,
  Q^T/K^T (head-dim on partitions) and V (token-dim on partitions) for its
  two heads via the QKV projection with its 768-column shard of Wqkv, runs
  causal attention locally (scores are computed transposed: S^T[k,q], so
  the softmax reduction over k maps to an all-ones matmul on the partition
  axis which also broadcasts the denominator), and produces att^T
  [256, 2048] per batch. A per-batch AllToAll redistributes from
  head-sharded to token-sharded: core c ends with att_all^T [2048, 256]
  for its token chunk of that batch, projects with the full Wout, and
  returns its token slices. The host concatenates the slices.

  Batch 0's attention is emitted interleaved with batch 1's projection so
  the PE fills the exp-latency gaps; batch 0's AllToAll and output
  projection overlap batch 1's attention.

All matmuls run in float32r (full PE rate at free-dim >= 256, ~1e-4 rel
error). PSUM accumulation is fp32.
"""

import numpy as np

import concourse.bass as bass
import concourse.mybir as mybir
import concourse.tile as tile
from concourse import bacc
from concourse.bass_utils import run_bass_kernel_spmd

B, T, D, H, Dh = 2, 2048, 2048, 16, 128
NT = B * T                  # 4096 tokens total
W = 8                       # cores
HL = H // W                 # 2 heads per core
CQKV = 3 * HL * Dh          # 768 qkv columns per core
KO = D // 128               # 16 contraction subtiles
TC = 256                    # token chunk for projection rhs
NTC_B = T // TC             # 8 chunks per batch
QC = 512                    # attention q-chunk
NQC = T // QC               # 4 q-chunks per batch
TOKB = T // W               # 256 tokens per core per batch after AllToAll
SCALE = 1.0 / float(np.sqrt(Dh))

F32 = mybir.dt.float32
F32R = mybir.dt.float32r


def _build():
    nc = bacc.Bacc("TRN2", target_bir_lowering=False, debug=False,
                   enable_asserts=True, num_devices=W)
    xT = nc.dram_tensor("xT", [D, NT], F32, kind="ExternalInput").ap()
    wqkv = nc.dram_tensor("wqkv", [D, CQKV], F32, kind="ExternalInput").ap()
    bqkv = nc.dram_tensor("bqkv", [CQKV], F32, kind="ExternalInput").ap()
    wout = nc.dram_tensor("wout", [D, D], F32, kind="ExternalInput").ap()
    bout = nc.dram_tensor("bout", [D], F32, kind="ExternalInput").ap()
    # mask[kl, j] = 1.0 if j >= kl else 0.0 (same triangle for every
    # diagonal 128-col sub-block)
    masktri = nc.dram_tensor("masktri", [128, 128], F32, kind="ExternalInput").ap()
    ones = nc.dram_tensor("ones", [128, 128], F32, kind="ExternalInput").ap()
    bvbc = nc.dram_tensor("bvbc", [128, HL * Dh], F32, kind="ExternalInput").ap()
    boutbc = nc.dram_tensor("boutbc", [128, D], F32, kind="ExternalInput").ap()
    # rows [b*TOKB, (b+1)*TOKB) = this core's tokens of batch b
    out = nc.dram_tensor("out", [B * TOKB, D], F32, kind="ExternalOutput").ap()

    xT_v = xT.rearrange("(ko p) t -> p ko t", p=128)
    wqkv_v = wqkv.rearrange("(ko p) c -> p ko c", p=128)
    wout_v = wout.rearrange("(ko p) c -> p ko c", p=128)

    with tile.TileContext(nc) as tc:
        with tc.tile_pool(name="persist", bufs=1) as persist, \
             tc.tile_pool(name="dram", bufs=1, space="DRAM") as dram_pool:
            mask_sb = persist.tile([128, 128], F32R)
            ones_sb = persist.tile([128, 128], F32R)
            bqk_sb = persist.tile([128, 2 * HL], F32)      # Q,K bias (col on partition)
            bv_sb = persist.tile([128, HL * Dh], F32)      # V bias pre-broadcast

            nc.sync.dma_start(mask_sb[:], masktri.bitcast(F32R))
            nc.sync.dma_start(ones_sb[:], ones.bitcast(F32R))
            nc.sync.dma_start(bqk_sb[:], bqkv[0:2 * HL * 128].rearrange("(cc p) -> p cc", p=128))
            nc.sync.dma_start(bv_sb[:], bvbc)

            a2a_in = [dram_pool.tile([W, HL * 128, TOKB], F32, name=f"a2a_in{b}")
                      for b in range(B)]
            a2a_out = [dram_pool.tile([W, HL * 128, TOKB], F32, name=f"a2a_out{b}")
                       for b in range(B)]

            # per-batch qkv tiles so batch 0's space frees before out-proj
            def alloc_qkv(pool):
                qT = pool.tile([128, HL, T], F32R, name="qT")
                kT = pool.tile([128, HL, T], F32R, name="kT")
                v = pool.tile([128, HL, T // 128, Dh], F32R, name="v")
                return qT, kT, v

            def emit_proj_chunk(qkv, wqkv_sb, x_pool, proj_psum, b, tci):
                """Project one 256-token chunk of batch b into (qT, kT, v)."""
                qT_sb, kT_sb, v_sb = qkv
                t0 = b * T + tci * TC
                x_sb = x_pool.tile([128, KO, TC], F32R, name="x_sb")
                nc.sync.dma_start(x_sb[:], xT_v[:, :, t0:t0 + TC].bitcast(F32R))
                for cc in range(2 * HL):
                    ps = proj_psum.tile([128, TC], F32, name="proj_ps")
                    for ko in range(KO):
                        nc.tensor.matmul(
                            ps[:], wqkv_sb[:, ko, cc * 128:(cc + 1) * 128],
                            x_sb[:, ko, :], start=(ko == 0), stop=(ko == KO - 1))
                    dest = qT_sb if cc < HL else kT_sb
                    hl = cc if cc < HL else cc - HL
                    nc.vector.tensor_scalar_add(
                        dest[:, hl, tci * TC:(tci + 1) * TC], ps[:],
                        bqk_sb[:, cc:cc + 1])
                for tb in range(TC // 128):
                    ps = proj_psum.tile([128, HL * Dh], F32, name="proj_ps")
                    for ko in range(KO):
                        nc.tensor.matmul(
                            ps[:], x_sb[:, ko, tb * 128:(tb + 1) * 128],
                            wqkv_sb[:, ko, 2 * HL * 128:], start=(ko == 0), stop=(ko == KO - 1))
                    idx = tci * (TC // 128) + tb
                    nc.vector.tensor_tensor(
                        v_sb[:, :, idx, :],
                        ps[:].rearrange("p (hl d) -> p hl d", hl=HL),
                        bv_sb[:].rearrange("p (hl d) -> p hl d", hl=HL),
                        mybir.AluOpType.add)

            def emit_attn_group(qkv, att_sb, pools, hl, qc):
                """One (head, q-chunk) attention group: S^T -> exp -> P^T V."""
                qT_sb, kT_sb, v_sb = qkv
                ex_pool, rden_pool, s_psum, av_psum, d_psum = pools
                q0 = qc * QC
                nkb = (qc + 1) * (QC // 128)
                ps_av = av_psum.tile([128, QC], F32, name="ps_av")
                ps_dbc = d_psum.tile([128, QC], F32, name="ps_dbc")
                for kb in range(nkb):
                    o = kb - qc * (QC // 128)
                    vs = max(0, o) * 128        # first valid q column
                    ps_s = s_psum.tile([128, QC], F32, name="ps_s")
                    nc.tensor.matmul(
                        ps_s[:, vs:], kT_sb[:, hl, kb * 128:(kb + 1) * 128],
                        qT_sb[:, hl, q0 + vs:q0 + QC], start=True, stop=True)
                    ex = ex_pool.tile([128, QC], F32R, name="ex")
                    nc.scalar.activation(
                        ex[:, vs:], ps_s[:, vs:], mybir.ActivationFunctionType.Exp,
                        scale=SCALE)
                    if o >= 0:
                        nc.vector.tensor_tensor(
                            ex[:, vs:vs + 128], ex[:, vs:vs + 128], mask_sb[:],
                            mybir.AluOpType.mult)
                    nc.tensor.matmul(
                        ps_av[:, vs:], v_sb[:, hl, kb, :], ex[:, vs:],
                        start=(kb == 0), stop=(kb == nkb - 1))
                    nc.tensor.matmul(
                        ps_dbc[:, vs:], ones_sb[:], ex[:, vs:],
                        start=(kb == 0), stop=(kb == nkb - 1))
                rden = rden_pool.tile([128, QC], F32, name="rden")
                nc.vector.reciprocal(rden[:], ps_dbc[:])
                nc.vector.tensor_tensor(
                    att_sb[:, hl, q0:q0 + QC], ps_av[:], rden[:],
                    mybir.AluOpType.mult)

            def emit_a2a(att_sb, b):
                for r in range(W):
                    nc.sync.dma_start(
                        a2a_in[b][r].rearrange("(hl p) t -> p hl t", hl=HL, p=128),
                        att_sb[:, :, r * TOKB:(r + 1) * TOKB])
                nc.gpsimd.collective_compute(
                    "AllToAll", mybir.AluOpType.bypass,
                    replica_groups=[list(range(W))],
                    ins=[a2a_in[b][:].opt()], outs=[a2a_out[b][:].opt()])

            def emit_outproj(attall_pool, wout_pool, o_pool, out_psum, bout_sb, b):
                attall_sb = attall_pool.tile([128, KO, TOKB], F32R, name="attall")
                nc.sync.dma_start(
                    attall_sb[:],
                    a2a_out[b][:].rearrange("r (x p) t -> p (r x) t", x=HL, p=128).bitcast(F32R))
                for colc in range(D // 512):
                    wout_sb = wout_pool.tile([128, KO, 512], F32R, name="wout_sb")
                    nc.sync.dma_start(
                        wout_sb[:], wout_v[:, :, colc * 512:(colc + 1) * 512].bitcast(F32R))
                    for tokc in range(TOKB // 128):
                        ps_o = out_psum.tile([128, 512], F32, name="ps_o")
                        for ko in range(KO):
                            nc.tensor.matmul(
                                ps_o[:], attall_sb[:, ko, tokc * 128:(tokc + 1) * 128],
                                wout_sb[:, ko, :], start=(ko == 0), stop=(ko == KO - 1))
                        o_sb = o_pool.tile([128, 512], F32, name="o_sb")
                        nc.vector.tensor_tensor(
                            o_sb[:], ps_o[:],
                            bout_sb[:, colc * 512:(colc + 1) * 512],
                            mybir.AluOpType.add)
                        nc.sync.dma_start(
                            out[b * TOKB + tokc * 128:b * TOKB + (tokc + 1) * 128,
                                colc * 512:(colc + 1) * 512],
                            o_sb[:])

            with tc.tile_pool(name="qkv1_pool", bufs=1) as qkv1_pool:
                qkv1 = alloc_qkv(qkv1_pool)
                with tc.tile_pool(name="qkv0_pool", bufs=1) as qkv0_pool:
                    qkv0 = alloc_qkv(qkv0_pool)
                    with tc.tile_pool(name="att0_pool", bufs=1) as att0_pool:
                        att0_sb = att0_pool.tile([128, HL, T], F32)
                        with tc.tile_pool(name="wq_pool", bufs=1) as wq_pool, \
                             tc.tile_pool(name="x_pool", bufs=2) as x_pool, \
                             tc.tile_pool(name="proj_psum", bufs=4, space="PSUM") as proj_psum, \
                             tc.tile_pool(name="ex0_pool", bufs=3) as ex0_pool, \
                             tc.tile_pool(name="rden0_pool", bufs=2) as rden0_pool, \
                             tc.tile_pool(name="s0_psum", bufs=2, space="PSUM") as s0_psum, \
                             tc.tile_pool(name="av0_psum", bufs=1, space="PSUM") as av0_psum, \
                             tc.tile_pool(name="d0_psum", bufs=1, space="PSUM") as d0_psum:
                            wqkv_sb = wq_pool.tile([128, KO, CQKV], F32R)
                            nc.sync.dma_start(wqkv_sb[:], wqkv_v.bitcast(F32R))
                            pools0 = (ex0_pool, rden0_pool, s0_psum, av0_psum, d0_psum)
                            # batch-0 projection
                            for tci in range(NTC_B):
                                emit_proj_chunk(qkv0, wqkv_sb, x_pool, proj_psum, 0, tci)
                            # batch-1 projection interleaved with batch-0 attention
                            groups0 = [(hl, qc) for hl in range(HL) for qc in range(NQC)]
                            for i in range(NTC_B):
                                emit_proj_chunk(qkv1, wqkv_sb, x_pool, proj_psum, 1, i)
                                emit_attn_group(qkv0, att0_sb, pools0, *groups0[i])
                        emit_a2a(att0_sb, 0)
                # batch-1 attention + overlap with A2A#0 / out-proj#0
                with tc.tile_pool(name="att1_pool", bufs=1) as att1_pool:
                    att1_sb = att1_pool.tile([128, HL, T], F32)
                    with tc.tile_pool(name="ex1_pool", bufs=3) as ex1_pool, \
                         tc.tile_pool(name="rden1_pool", bufs=2) as rden1_pool, \
                         tc.tile_pool(name="s1_psum", bufs=2, space="PSUM") as s1_psum, \
                         tc.tile_pool(name="av1_psum", bufs=2, space="PSUM") as av1_psum, \
                         tc.tile_pool(name="d1_psum", bufs=2, space="PSUM") as d1_psum, \
                         tc.tile_pool(name="attall_pool", bufs=2) as attall_pool, \
                         tc.tile_pool(name="wout_pool", bufs=2) as wout_pool, \
                         tc.tile_pool(name="o_pool", bufs=3) as o_pool, \
                         tc.tile_pool(name="out_psum", bufs=2, space="PSUM") as out_psum:
                        bout_sb = attall_pool.tile([128, D], F32, name="bout_sb", bufs=1)
                        nc.sync.dma_start(bout_sb[:], boutbc)
                        pools1 = (ex1_pool, rden1_pool, s1_psum, av1_psum, d1_psum)
                        for hl in range(HL):
                            for qc in range(NQC):
                                emit_attn_group(qkv1, att1_sb, pools1, hl, qc)
                        emit_a2a(att1_sb, 1)
                        emit_outproj(attall_pool, wout_pool, o_pool, out_psum, bout_sb, 0)
                        emit_outproj(attall_pool, wout_pool, o_pool, out_psum, bout_sb, 1)
    nc.compile()
    return nc


_CACHED_NC = None


def kernel(x, Wqkv, bqkv, Wout, bout):
    global _CACHED_NC
    x = np.asarray(x, dtype=np.float32)
    Wqkv = np.asarray(Wqkv, dtype=np.float32)
    bqkv = np.asarray(bqkv, dtype=np.float32)
    Wout = np.asarray(Wout, dtype=np.float32)
    bout = np.asarray(bout, dtype=np.float32)

    if _CACHED_NC is None:
        _CACHED_NC = _build()
    nc = _CACHED_NC

    xT = np.ascontiguousarray(x.reshape(NT, D).T)          # [D, NT]
    wq4 = Wqkv.reshape(D, 3, H, Dh)                        # col = which, head, dh
    bq4 = bqkv.reshape(3, H, Dh)
    kl = np.arange(128)[:, None]
    jl = np.arange(128)[None, :]
    masktri = (jl >= kl).astype(np.float32)

    in_maps = []
    for c in range(W):
        wshard = np.ascontiguousarray(
            wq4[:, :, HL * c:HL * c + HL, :].reshape(D, CQKV))
        bshard = np.ascontiguousarray(
            bq4[:, HL * c:HL * c + HL, :].reshape(CQKV))
        in_maps.append({
            "xT": xT, "wqkv": wshard, "bqkv": bshard,
            "wout": Wout, "bout": bout, "masktri": masktri,
            "ones": np.ones((128, 128), np.float32),
            "bvbc": np.tile(bshard[2 * HL * 128:][None, :], (128, 1)),
            "boutbc": np.tile(bout[None, :], (128, 1)),
        })

    res = run_bass_kernel_spmd(nc, in_maps, core_ids=list(range(W)))
    # res[c]["out"] rows: [b*TOKB:(b+1)*TOKB) = tokens [c*TOKB,(c+1)*TOKB) of batch b
    full = np.empty((B, T, D), np.float32)
    for c in range(W):
        for b in range(B):
            full[b, c * TOKB:(c + 1) * TOKB] = res.results[c]["out"][b * TOKB:(b + 1) * TOKB]
    return full
